# revision 4
# baseline (speedup 1.0000x reference)
"""DeStationaryAttention Trainium2 kernel (bf16 + DMA-transpose pipeline).

Full inputs in, full output out. Sharding: B*N = 64 attention heads are
split across 8 NeuronCores, 8 heads each: core c handles batch b = c//2,
nodes n0 = (c%2)*8 .. n0+8. Inputs are pre-sliced on the host so each
core receives contiguous [T=1024, H=8, D=128] tensors; Q/K/V are cast
to bf16 on the host (pure dtype staging — all arithmetic runs on HW).

Per-head math (T=1024, D=128):
    Qc = Q - mean_T(Q)
    tau = 2*sigmoid(mean_T(std)*w + b)          (scalar per head)
    S[t,s] = Qc[t]·K[s] / sqrt(D)               (K-centering dropped:
             softmax_s(Qc·(K-muK)) == softmax_s(Qc·K), row-constant)
    out = softmax(tau*S) @ V

Engine assignment (keeps the Scalar engine exp-only — it is the wall):
    DMA-XBAR : qT/kT loaded transposed straight from HBM (bf16 crossbar
               transpose, verified layout); O^T -> natural also via XBAR
               (row fold t = tt*128 + p, verified)
    DVE      : muQ reduce + centering add, esum adds (two partial
               chains), O^T evac cast, reciprocal, 1/rowsum scales
    ScalarE  : exp(tau_scale * S^T) PSUM -> SBUF bf16 (64 ACTs)
    PE       : S^T / O^T matmuls (bf16, 1 cyc/row) + rowsum mini-matmuls
               (two-step PSUM accumulation over the esum partials)

Emission is software-pipelined (prep(h+1) before finalize(h), finalize
interleaved one s-tile into sloop(h+1)) so engine queues overlap heads.
"""

import os
import sys
from contextlib import ExitStack

for _p in ("/root/.axon_site/_ro/trn_rl_repo", "/opt/trn_rl_repo"):
    if os.path.isdir(_p) and _p not in sys.path:
        sys.path.append(_p)

import numpy as np
import ml_dtypes

import concourse.bass as bass
import concourse.mybir as mybir
import concourse.tile as tile
from concourse import bacc
from concourse.bass_utils import run_bass_kernel_spmd

B, T, N, D = 4, 1024, 16, 128
H = 8           # heads per core
NCORES = 8
TT = T // 128   # 128-row tiles along T
F32 = mybir.dt.float32
BF16 = mybir.dt.bfloat16
SCALE2 = 2.0 * D ** (-0.5)   # folded 2*sigmoid(...) * D^-0.5 broadcast constant


def _emit(tc):
    nc = tc.nc
    q_d = nc.dram_tensor("Q", [T, H, D], BF16, kind="ExternalInput").ap()
    k_d = nc.dram_tensor("K", [T, H, D], BF16, kind="ExternalInput").ap()
    v_d = nc.dram_tensor("V", [T, H, D], BF16, kind="ExternalInput").ap()
    std_d = nc.dram_tensor("S", [T, H], F32, kind="ExternalInput").ap()
    tw_d = nc.dram_tensor("TW", [1, 1], F32, kind="ExternalInput").ap()
    tb_d = nc.dram_tensor("TB", [1, 1], F32, kind="ExternalInput").ap()
    o_d = nc.dram_tensor("O", [T, H, D], F32, kind="ExternalOutput").ap()

    Exp = mybir.ActivationFunctionType.Exp
    X = mybir.AxisListType.X

    ctx = ExitStack()
    const = ctx.enter_context(tc.tile_pool(name="const", bufs=1))
    nat = ctx.enter_context(tc.tile_pool(name="nat", bufs=2))
    big = ctx.enter_context(tc.tile_pool(name="big", bufs=4))
    qrawp = ctx.enter_context(tc.tile_pool(name="qrawp", bufs=2))
    etp = ctx.enter_context(tc.tile_pool(name="etp", bufs=6))
    esp = ctx.enter_context(tc.tile_pool(name="esp", bufs=4))
    otsp = ctx.enter_context(tc.tile_pool(name="otsp", bufs=2))
    on16p = ctx.enter_context(tc.tile_pool(name="on16p", bufs=2))
    onatp = ctx.enter_context(tc.tile_pool(name="onatp", bufs=2))
    smallp = ctx.enter_context(tc.tile_pool(name="smallp", bufs=4))
    ps_st = ctx.enter_context(tc.tile_pool(name="ps_st", bufs=2, space="PSUM"))
    ps_ot = ctx.enter_context(tc.tile_pool(name="ps_ot", bufs=1, space="PSUM"))
    ps_sm = ctx.enter_context(tc.tile_pool(name="ps_sm", bufs=2, space="PSUM"))

    # constants
    ones16 = const.tile([128, 1], BF16)
    nc.vector.memset(ones16, 1.0)
    inv_t = const.tile([128, 1], F32)
    nc.vector.memset(inv_t, 1.0 / T)
    bc2 = const.tile([1, 128], F32)
    nc.vector.memset(bc2, SCALE2)

    std_sb = const.tile([128, T * H // 128], F32)   # [128, 64] contiguous
    nc.sync.dma_start(out=std_sb, in_=std_d.rearrange("(p j) h -> p (j h)", p=128))
    tw_sb = const.tile([1, 1], F32)
    nc.sync.dma_start(out=tw_sb, in_=tw_d)
    tb_sb = const.tile([1, 1], F32)
    nc.sync.dma_start(out=tb_sb, in_=tb_d)
    negw = const.tile([1, 1], F32)
    nc.vector.tensor_scalar_mul(negw, tw_sb, -1.0)
    negb = const.tile([1, 1], F32)
    nc.vector.tensor_scalar_mul(negb, tb_sb, -1.0)

    std3 = std_sb.rearrange("p (j h) -> p j h", h=H)

    # ---- tau prologue (batched over heads: 2 PE matmuls total) ----
    taup = ctx.enter_context(tc.tile_pool(name="taup", bufs=1))
    tau_scs = []

    def emit_taus():
        part_all = smallp.tile([128, H], F32, tag="part")
        for h in range(H):
            nc.vector.reduce_sum(out=part_all[:, h:h + 1], in_=std3[:, :, h], axis=X)
        mean_ps = ps_sm.tile([1, H], F32, tag="ps_sm")
        nc.tensor.matmul(mean_ps, lhsT=inv_t, rhs=part_all, start=True, stop=True)
        ez = smallp.tile([1, H], F32, tag="ez")
        nc.scalar.activation(ez, mean_ps, Exp, bias=negb[:], scale=negw[:])
        den = smallp.tile([1, H], F32, tag="den")
        nc.vector.tensor_scalar_add(den, ez, 1.0)
        sig = smallp.tile([1, H], F32, tag="sig")
        nc.vector.reciprocal(sig, den)
        tau_ps = ps_sm.tile([128, H], F32, tag="ps_sm")
        nc.tensor.matmul(tau_ps, lhsT=bc2, rhs=sig, start=True, stop=True)
        tau_all = taup.tile([128, H], F32)
        nc.vector.tensor_copy(tau_all, tau_ps)
        for h in range(H):
            tau_scs.append(tau_all[:, h:h + 1])

    def prep(h):
        # transposed loads straight from HBM via the DMA crossbar
        qT = qrawp.tile([128, T], BF16, tag="qT")
        nc.sync.dma_start_transpose(qT, q_d[:, h, :])
        kT = big.tile([128, T], BF16, tag="kT")
        nc.sync.dma_start_transpose(kT, k_d[:, h, :])
        v16 = nat.tile([128, TT, 128], BF16, tag="v16")
        nc.sync.dma_start(out=v16, in_=v_d[:, h, :].rearrange("(tt p) d -> p tt d", p=128))

        qsum = smallp.tile([128, 1], F32, tag="qsum")
        nc.vector.reduce_sum(out=qsum, in_=qT, axis=X)
        nmu = smallp.tile([128, 1], F32, tag="nmu")
        nc.vector.tensor_scalar_mul(nmu, qsum, -1.0 / T)
        qcT = big.tile([128, T], BF16, tag="qcT")
        nc.vector.tensor_scalar_add(qcT, qT, nmu)
        return {"qcT": qcT, "kT": kT, "v16": v16}

    def sloop(h, st, lo=0, hi=TT):
        qcT, kT, v16 = st["qcT"], st["kT"], st["v16"]
        tau_sc = tau_scs[h]
        if lo == 0:
            st["ot_ps"] = ps_ot.tile([128, T], F32, tag="ps_ot", name="ot_ps")
            st["esumA"] = esp.tile([128, T], BF16, tag="esumA", name="esumA")
            st["esumB"] = esp.tile([128, T], BF16, tag="esumB", name="esumB")
            st["prev_et"] = None
        ot_ps = st["ot_ps"]
        prev_et = st["prev_et"]
        def emit_av(i, et):
            vlhs = v16[:, i, :]
            nc.tensor.matmul(ot_ps[:, 0:512], lhsT=vlhs, rhs=et[:, 0:512], start=(i == 0), stop=(i == TT - 1))
            nc.tensor.matmul(ot_ps[:, 512:1024], lhsT=vlhs, rhs=et[:, 512:1024], start=(i == 0), stop=(i == TT - 1))

        # in-loop software pipeline: S-matmuls of tile i are emitted before the
        # AV-matmuls of tile i-1, so the PE queue never parks on an AV whose
        # exp hasn't finished while the next S could run.
        pend = st.get("pend_av") or []
        for i in range(lo, hi):
            st_ps = ps_st.tile([128, T], F32, tag="ps_st")
            klhs = kT[:, i * 128:(i + 1) * 128]
            nc.tensor.matmul(st_ps[:, 0:512], lhsT=klhs, rhs=qcT[:, 0:512], start=True, stop=True)
            nc.tensor.matmul(st_ps[:, 512:1024], lhsT=klhs, rhs=qcT[:, 512:1024], start=True, stop=True)
            et = etp.tile([128, T], BF16, tag="et")
            nc.scalar.activation(et, st_ps, Exp, bias=0.0, scale=tau_sc[:])
            pend.append((i, et))
            if len(pend) > 2:
                emit_av(*pend.pop(0))
            # two independent esum chains (halves DVE serial latency and
            # lets the rowsum minis accumulate over both in PSUM)
            esum = st["esumA"] if i < 4 else st["esumB"]
            if i % 4 == 1:
                nc.vector.tensor_add(esum, prev_et, et)
            elif i % 4 > 1:
                nc.vector.tensor_add(esum, esum, et)
            prev_et = et
        if hi == TT:
            while pend:
                emit_av(*pend.pop(0))
        st["pend_av"] = pend
        st["prev_et"] = prev_et

    def finalize(h, st):
        esumA, esumB, ot_ps = st["esumA"], st["esumB"], st["ot_ps"]
        # XBAR output transpose folds t = tt*128 + p (verified), matching the
        # contiguous esum column slices below.
        rs_ps = ps_sm.tile([128, TT], F32, tag="ps_sm")
        for tt in range(TT):
            sl = slice(tt * 128, (tt + 1) * 128)
            nc.tensor.matmul(rs_ps[:, tt:tt + 1], lhsT=esumA[:, sl],
                             rhs=ones16, start=True, stop=False)
            nc.tensor.matmul(rs_ps[:, tt:tt + 1], lhsT=esumB[:, sl],
                             rhs=ones16, start=False, stop=True)
        recipT = smallp.tile([128, TT], F32, tag="recipT")
        nc.vector.reciprocal(recipT, rs_ps)

        ots = otsp.tile([128, T], BF16, tag="ots")
        nc.vector.tensor_copy(ots, ot_ps)
        on16 = on16p.tile([128, TT, 128], BF16, tag="on16")
        nc.sync.dma_start_transpose(on16, ots)
        o_nat = onatp.tile([128, TT, 128], F32, tag="o_nat")
        for tt in range(TT):
            nc.vector.tensor_scalar_mul(o_nat[:, tt, :], on16[:, tt, :],
                                        recipT[:, tt:tt + 1])
        nc.sync.dma_start(out=o_d[:, h, :].rearrange("(tt p) d -> p tt d", p=128), in_=o_nat)

    # software-pipelined emission: head h+1's prep lands on each engine's
    # queue BEFORE head h's finalize, so the inter-head transpose/centering
    # chain overlaps the previous head's tail instead of serializing after it.
    states = [None] * H
    emit_taus()
    states[0] = prep(0)
    sloop(0, states[0])
    for h in range(1, H):
        states[h] = prep(h)
        sloop(h, states[h], 0, 1)
        finalize(h - 1, states[h - 1])
        sloop(h, states[h], 1, TT)
    finalize(H - 1, states[H - 1])
    ctx.close()


_BUILT = None


def _build():
    global _BUILT
    if _BUILT is None:
        nc = bacc.Bacc("TRN2", target_bir_lowering=False, debug=False, num_devices=None)
        with tile.TileContext(nc) as tc:
            _emit(tc)
        nc.compile()
        _BUILT = nc
    return _BUILT


def _in_maps(Q, K, V, std, tau_w, tau_b):
    bf = ml_dtypes.bfloat16
    tw = np.asarray(tau_w, np.float32).reshape(1, 1)
    tb = np.asarray(tau_b, np.float32).reshape(1, 1)
    maps = []
    for c in range(NCORES):
        b, n0 = c // 2, (c % 2) * H
        maps.append({
            "Q": np.ascontiguousarray(Q[b, :, n0:n0 + H, :]).astype(bf),
            "K": np.ascontiguousarray(K[b, :, n0:n0 + H, :]).astype(bf),
            "V": np.ascontiguousarray(V[b, :, n0:n0 + H, :]).astype(bf),
            "S": np.ascontiguousarray(std[b, :, n0:n0 + H, 0], np.float32),
            "TW": tw,
            "TB": tb,
        })
    return maps


def _gather(results):
    out = np.empty((B, T, N, D), np.float32)
    for c in range(NCORES):
        b, n0 = c // 2, (c % 2) * H
        out[b, :, n0:n0 + H, :] = results[c]["O"]
    return out


def run(Q, K, V, std, tau_w, tau_b, **spmd_kwargs):
    nc = _build()
    res = run_bass_kernel_spmd(nc, _in_maps(Q, K, V, std, tau_w, tau_b),
                               core_ids=list(range(NCORES)), **spmd_kwargs)
    return _gather(res.results), res


def kernel(Q, K, V, std, tau_w, tau_b):
    out, _ = run(Q, K, V, std, tau_w, tau_b)
    return out


# revision 6
# speedup vs baseline: 1.0263x; 1.0263x over previous
"""DeStationaryAttention Trainium2 kernel (bf16 + DMA-transpose pipeline).

Full inputs in, full output out. Sharding: B*N = 64 attention heads are
split across 8 NeuronCores, 8 heads each: core c handles batch b = c//2,
nodes n0 = (c%2)*8 .. n0+8. Inputs are pre-sliced on the host so each
core receives contiguous [T=1024, H=8, D=128] tensors; Q/K/V are cast
to bf16 on the host (pure dtype staging — all arithmetic runs on HW).

Per-head math (T=1024, D=128):
    Qc = Q - mean_T(Q)
    tau = 2*sigmoid(mean_T(std)*w + b)          (scalar per head)
    S[t,s] = Qc[t]·K[s] / sqrt(D)               (K-centering dropped:
             softmax_s(Qc·(K-muK)) == softmax_s(Qc·K), row-constant)
    out = softmax(tau*S) @ V

Engine assignment (keeps the Scalar engine exp-only — it is the wall):
    DMA-XBAR : qT/kT loaded transposed straight from HBM (bf16 crossbar
               transpose, verified layout); O^T -> natural also via XBAR
               (row fold t = tt*128 + p, verified)
    DVE      : muQ reduce + centering add, esum adds (two partial
               chains), O^T evac cast, reciprocal, 1/rowsum scales
    ScalarE  : exp(tau_scale * S^T) PSUM -> SBUF bf16 (64 ACTs)
    PE       : S^T / O^T matmuls (bf16, 1 cyc/row) + rowsum mini-matmuls
               (two-step PSUM accumulation over the esum partials)

Emission is software-pipelined (prep(h+1) before finalize(h), finalize
interleaved one s-tile into sloop(h+1)) so engine queues overlap heads.
"""

import os
import sys
from contextlib import ExitStack

for _p in ("/root/.axon_site/_ro/trn_rl_repo", "/opt/trn_rl_repo"):
    if os.path.isdir(_p) and _p not in sys.path:
        sys.path.append(_p)

import numpy as np
import ml_dtypes

import concourse.bass as bass
import concourse.mybir as mybir
import concourse.tile as tile
from concourse import bacc
from concourse.bass_utils import run_bass_kernel_spmd

B, T, N, D = 4, 1024, 16, 128
H = 8           # heads per core
NCORES = 8
TT = T // 128   # 128-row tiles along T
F32 = mybir.dt.float32
BF16 = mybir.dt.bfloat16
SCALE2 = 2.0 * D ** (-0.5)   # folded 2*sigmoid(...) * D^-0.5 broadcast constant


def _emit(tc):
    nc = tc.nc
    q_d = nc.dram_tensor("Q", [T, H, D], BF16, kind="ExternalInput").ap()
    k_d = nc.dram_tensor("K", [T, H, D], BF16, kind="ExternalInput").ap()
    v_d = nc.dram_tensor("V", [T, H, D], BF16, kind="ExternalInput").ap()
    std_d = nc.dram_tensor("S", [T, H], F32, kind="ExternalInput").ap()
    tw_d = nc.dram_tensor("TW", [1, 1], F32, kind="ExternalInput").ap()
    tb_d = nc.dram_tensor("TB", [1, 1], F32, kind="ExternalInput").ap()
    o_d = nc.dram_tensor("O", [T, H, D], F32, kind="ExternalOutput").ap()

    Exp = mybir.ActivationFunctionType.Exp
    X = mybir.AxisListType.X

    ctx = ExitStack()
    const = ctx.enter_context(tc.tile_pool(name="const", bufs=1))
    nat = ctx.enter_context(tc.tile_pool(name="nat", bufs=3))
    big = ctx.enter_context(tc.tile_pool(name="big", bufs=6))
    qrawp = ctx.enter_context(tc.tile_pool(name="qrawp", bufs=3))
    etp = ctx.enter_context(tc.tile_pool(name="etp", bufs=6))
    esp = ctx.enter_context(tc.tile_pool(name="esp", bufs=4))
    otsp = ctx.enter_context(tc.tile_pool(name="otsp", bufs=2))
    on16p = ctx.enter_context(tc.tile_pool(name="on16p", bufs=2))
    onatp = ctx.enter_context(tc.tile_pool(name="onatp", bufs=2))
    smallp = ctx.enter_context(tc.tile_pool(name="smallp", bufs=4))
    ps_st = ctx.enter_context(tc.tile_pool(name="ps_st", bufs=2, space="PSUM"))
    ps_ot = ctx.enter_context(tc.tile_pool(name="ps_ot", bufs=1, space="PSUM"))
    ps_sm = ctx.enter_context(tc.tile_pool(name="ps_sm", bufs=2, space="PSUM"))

    # constants
    ones16 = const.tile([128, 1], BF16)
    nc.vector.memset(ones16, 1.0)
    inv_t = const.tile([128, 1], F32)
    nc.vector.memset(inv_t, 1.0 / T)
    bc2 = const.tile([1, 128], F32)
    nc.vector.memset(bc2, SCALE2)

    std_sb = const.tile([128, T * H // 128], F32)   # [128, 64] contiguous
    nc.sync.dma_start(out=std_sb, in_=std_d.rearrange("(p j) h -> p (j h)", p=128))
    tw_sb = const.tile([1, 1], F32)
    nc.sync.dma_start(out=tw_sb, in_=tw_d)
    tb_sb = const.tile([1, 1], F32)
    nc.sync.dma_start(out=tb_sb, in_=tb_d)
    negw = const.tile([1, 1], F32)
    nc.vector.tensor_scalar_mul(negw, tw_sb, -1.0)
    negb = const.tile([1, 1], F32)
    nc.vector.tensor_scalar_mul(negb, tb_sb, -1.0)

    std3 = std_sb.rearrange("p (j h) -> p j h", h=H)

    # ---- tau prologue (batched over heads: 2 PE matmuls total) ----
    taup = ctx.enter_context(tc.tile_pool(name="taup", bufs=1))
    tau_scs = []

    def emit_taus():
        part_all = smallp.tile([128, H], F32, tag="part")
        for h in range(H):
            nc.vector.reduce_sum(out=part_all[:, h:h + 1], in_=std3[:, :, h], axis=X)
        mean_ps = ps_sm.tile([1, H], F32, tag="ps_sm")
        nc.tensor.matmul(mean_ps, lhsT=inv_t, rhs=part_all, start=True, stop=True)
        ez = smallp.tile([1, H], F32, tag="ez")
        nc.scalar.activation(ez, mean_ps, Exp, bias=negb[:], scale=negw[:])
        den = smallp.tile([1, H], F32, tag="den")
        nc.vector.tensor_scalar_add(den, ez, 1.0)
        sig = smallp.tile([1, H], F32, tag="sig")
        nc.vector.reciprocal(sig, den)
        tau_ps = ps_sm.tile([128, H], F32, tag="ps_sm")
        nc.tensor.matmul(tau_ps, lhsT=bc2, rhs=sig, start=True, stop=True)
        tau_all = taup.tile([128, H], F32)
        nc.vector.tensor_copy(tau_all, tau_ps)
        for h in range(H):
            tau_scs.append(tau_all[:, h:h + 1])

    def prep(h):
        # transposed loads straight from HBM via the DMA crossbar
        qT = qrawp.tile([128, T], BF16, tag="qT")
        nc.sync.dma_start_transpose(qT, q_d[:, h, :])
        kT = big.tile([128, T], BF16, tag="kT")
        nc.sync.dma_start_transpose(kT, k_d[:, h, :])
        v16 = nat.tile([128, TT, 128], BF16, tag="v16")
        nc.sync.dma_start(out=v16, in_=v_d[:, h, :].rearrange("(tt p) d -> p tt d", p=128))

        qsum = smallp.tile([128, 1], F32, tag="qsum")
        nc.vector.reduce_sum(out=qsum, in_=qT, axis=X)
        nmu = smallp.tile([128, 1], F32, tag="nmu")
        nc.vector.tensor_scalar_mul(nmu, qsum, -1.0 / T)
        qcT = big.tile([128, T], BF16, tag="qcT")
        nc.vector.tensor_scalar_add(qcT, qT, nmu)
        return {"qcT": qcT, "kT": kT, "v16": v16}

    def sloop(h, st, lo=0, hi=TT):
        qcT, kT, v16 = st["qcT"], st["kT"], st["v16"]
        tau_sc = tau_scs[h]
        if lo == 0:
            st["ot_ps"] = ps_ot.tile([128, T], F32, tag="ps_ot", name="ot_ps")
            st["esumA"] = esp.tile([128, T], BF16, tag="esumA", name="esumA")
            st["esumB"] = esp.tile([128, T], BF16, tag="esumB", name="esumB")
            st["prev_et"] = None
        ot_ps = st["ot_ps"]
        prev_et = st["prev_et"]
        def emit_av(i, et):
            vlhs = v16[:, i, :]
            nc.tensor.matmul(ot_ps[:, 0:512], lhsT=vlhs, rhs=et[:, 0:512], start=(i == 0), stop=(i == TT - 1))
            nc.tensor.matmul(ot_ps[:, 512:1024], lhsT=vlhs, rhs=et[:, 512:1024], start=(i == 0), stop=(i == TT - 1))

        # in-loop software pipeline: S-matmuls of tile i are emitted before the
        # AV-matmuls of tile i-1, so the PE queue never parks on an AV whose
        # exp hasn't finished while the next S could run.
        pend = st.get("pend_av") or []
        for i in range(lo, hi):
            st_ps = ps_st.tile([128, T], F32, tag="ps_st")
            klhs = kT[:, i * 128:(i + 1) * 128]
            nc.tensor.matmul(st_ps[:, 0:512], lhsT=klhs, rhs=qcT[:, 0:512], start=True, stop=True)
            nc.tensor.matmul(st_ps[:, 512:1024], lhsT=klhs, rhs=qcT[:, 512:1024], start=True, stop=True)
            et = etp.tile([128, T], BF16, tag="et")
            nc.scalar.activation(et, st_ps, Exp, bias=0.0, scale=tau_sc[:])
            pend.append((i, et))
            if len(pend) > 2:
                emit_av(*pend.pop(0))
            # two independent esum chains (halves DVE serial latency and
            # lets the rowsum minis accumulate over both in PSUM)
            esum = st["esumA"] if i < 4 else st["esumB"]
            if i % 4 == 1:
                nc.vector.tensor_add(esum, prev_et, et)
            elif i % 4 > 1:
                nc.vector.tensor_add(esum, esum, et)
            prev_et = et
        if hi == TT:
            while pend:
                emit_av(*pend.pop(0))
        st["pend_av"] = pend
        st["prev_et"] = prev_et

    def finalize(h, st):
        esumA, esumB, ot_ps = st["esumA"], st["esumB"], st["ot_ps"]
        # XBAR output transpose folds t = tt*128 + p (verified), matching the
        # contiguous esum column slices below.
        rs_ps = ps_sm.tile([128, TT], F32, tag="ps_sm")
        for tt in range(TT):
            sl = slice(tt * 128, (tt + 1) * 128)
            nc.tensor.matmul(rs_ps[:, tt:tt + 1], lhsT=esumA[:, sl],
                             rhs=ones16, start=True, stop=False)
            nc.tensor.matmul(rs_ps[:, tt:tt + 1], lhsT=esumB[:, sl],
                             rhs=ones16, start=False, stop=True)
        recipT = smallp.tile([128, TT], F32, tag="recipT")
        nc.vector.reciprocal(recipT, rs_ps)

        ots = otsp.tile([128, T], BF16, tag="ots")
        nc.vector.tensor_copy(ots, ot_ps)
        on16 = on16p.tile([128, TT, 128], BF16, tag="on16")
        nc.sync.dma_start_transpose(on16, ots)
        o_nat = onatp.tile([128, TT, 128], F32, tag="o_nat")
        for tt in range(TT):
            nc.vector.tensor_scalar_mul(o_nat[:, tt, :], on16[:, tt, :],
                                        recipT[:, tt:tt + 1])
        nc.sync.dma_start(out=o_d[:, h, :].rearrange("(tt p) d -> p tt d", p=128), in_=o_nat)

    # software-pipelined emission, prep depth 2: head h+2's prep (XBAR
    # transposes + centering chain) is issued BEFORE head h's finalize DMAs.
    # The Sync queue is in-order, and finalize's output-transpose blocks on
    # that head's full compute — input transposes must be queued ahead of it
    # or every head transition stalls the Scalar (exp) stream.
    states = [None] * H
    states[0] = prep(0)
    states[1] = prep(1)
    emit_taus()
    sloop(0, states[0])
    for h in range(1, H):
        if h + 1 < H:
            states[h + 1] = prep(h + 1)
        sloop(h, states[h], 0, 1)
        finalize(h - 1, states[h - 1])
        sloop(h, states[h], 1, TT)
    finalize(H - 1, states[H - 1])
    ctx.close()


_BUILT = None


def _build():
    global _BUILT
    if _BUILT is None:
        nc = bacc.Bacc("TRN2", target_bir_lowering=False, debug=False, num_devices=None)
        with tile.TileContext(nc) as tc:
            _emit(tc)
        nc.compile()
        _BUILT = nc
    return _BUILT


def _in_maps(Q, K, V, std, tau_w, tau_b):
    bf = ml_dtypes.bfloat16
    tw = np.asarray(tau_w, np.float32).reshape(1, 1)
    tb = np.asarray(tau_b, np.float32).reshape(1, 1)
    maps = []
    for c in range(NCORES):
        b, n0 = c // 2, (c % 2) * H
        maps.append({
            "Q": np.ascontiguousarray(Q[b, :, n0:n0 + H, :]).astype(bf),
            "K": np.ascontiguousarray(K[b, :, n0:n0 + H, :]).astype(bf),
            "V": np.ascontiguousarray(V[b, :, n0:n0 + H, :]).astype(bf),
            "S": np.ascontiguousarray(std[b, :, n0:n0 + H, 0], np.float32),
            "TW": tw,
            "TB": tb,
        })
    return maps


def _gather(results):
    out = np.empty((B, T, N, D), np.float32)
    for c in range(NCORES):
        b, n0 = c // 2, (c % 2) * H
        out[b, :, n0:n0 + H, :] = results[c]["O"]
    return out


def run(Q, K, V, std, tau_w, tau_b, **spmd_kwargs):
    nc = _build()
    res = run_bass_kernel_spmd(nc, _in_maps(Q, K, V, std, tau_w, tau_b),
                               core_ids=list(range(NCORES)), **spmd_kwargs)
    return _gather(res.results), res


def kernel(Q, K, V, std, tau_w, tau_b):
    out, _ = run(Q, K, V, std, tau_w, tau_b)
    return out


# revision 13
# speedup vs baseline: 1.2295x; 1.1980x over previous
"""DeStationaryAttention Trainium2 kernel (bf16 + DMA-transpose pipeline).

Full inputs in, full output out. Sharding: B*N = 64 attention heads are
split across 8 NeuronCores, 8 heads each: core c handles batch b = c//2,
nodes n0 = (c%2)*8 .. n0+8. Inputs are pre-sliced on the host so each
core receives contiguous [T=1024, H=8, D=128] tensors; Q/K/V are cast
to bf16 on the host (pure dtype staging — all arithmetic runs on HW).

Per-head math (T=1024, D=128):
    Qc = Q - mean_T(Q)
    tau = 2*sigmoid(mean_T(std)*w + b)          (scalar per head)
    S[t,s] = Qc[t]·K[s] / sqrt(D)               (K-centering dropped:
             softmax_s(Qc·(K-muK)) == softmax_s(Qc·K), row-constant)
    out = softmax(tau*S) @ V

Engine assignment (keeps the Scalar engine exp-only — it is the wall):
    DMA-XBAR : qT/kT loaded transposed straight from HBM (bf16 crossbar
               transpose, verified layout). Output transposes stay on the
               PE: an XBAR output transpose is data-blocked on its head's
               full compute and head-of-line blocks the next heads' input
               transposes on the in-order Sync queue (measured 3-10us
               Scalar stalls per head boundary).
    DVE      : muQ reduce + centering add, esum adds (two partial
               chains), O^T evac cast, reciprocal, 1/rowsum scales
    ScalarE  : exp(tau_scale * S^T) PSUM -> SBUF bf16 (64 ACTs)
    PE       : S^T / O^T matmuls (bf16, 1 cyc/row), rowsum mini-matmuls
               (two-step PSUM accumulation over the esum partials), and
               output transposes (bf16 PSUM)
Output is stored bf16 and upcast on the host (rel_max 1.1e-2 vs fp32
reference, gate 2e-2).

Emission is software-pipelined (prep(h+1) before finalize(h), finalize
interleaved one s-tile into sloop(h+1)) so engine queues overlap heads.
"""

import os
import sys
from contextlib import ExitStack

for _p in ("/root/.axon_site/_ro/trn_rl_repo", "/opt/trn_rl_repo"):
    if os.path.isdir(_p) and _p not in sys.path:
        sys.path.append(_p)

import numpy as np
import ml_dtypes

import concourse.bass as bass
import concourse.mybir as mybir
import concourse.tile as tile
from concourse import bacc
from concourse.bass_utils import run_bass_kernel_spmd
from concourse.masks import make_identity

B, T, N, D = 4, 1024, 16, 128
H = 8           # heads per core
NCORES = 8
TT = T // 128   # 128-row tiles along T
F32 = mybir.dt.float32
BF16 = mybir.dt.bfloat16
SCALE2 = 2.0 * D ** (-0.5)   # folded 2*sigmoid(...) * D^-0.5 broadcast constant


def _emit(tc):
    nc = tc.nc
    q_d = nc.dram_tensor("Q", [T, H, D], BF16, kind="ExternalInput").ap()
    k_d = nc.dram_tensor("K", [T, H, D], BF16, kind="ExternalInput").ap()
    v_d = nc.dram_tensor("V", [T, H, D], BF16, kind="ExternalInput").ap()
    std_d = nc.dram_tensor("S", [T, H], F32, kind="ExternalInput").ap()
    tw_d = nc.dram_tensor("TW", [1, 1], F32, kind="ExternalInput").ap()
    tb_d = nc.dram_tensor("TB", [1, 1], F32, kind="ExternalInput").ap()
    o_d = nc.dram_tensor("O", [T, H, D], BF16, kind="ExternalOutput").ap()

    Exp = mybir.ActivationFunctionType.Exp
    X = mybir.AxisListType.X

    ctx = ExitStack()
    const = ctx.enter_context(tc.tile_pool(name="const", bufs=1))
    nat = ctx.enter_context(tc.tile_pool(name="nat", bufs=3))
    big = ctx.enter_context(tc.tile_pool(name="big", bufs=6))
    qrawp = ctx.enter_context(tc.tile_pool(name="qrawp", bufs=3))
    etp = ctx.enter_context(tc.tile_pool(name="etp", bufs=6))
    esp = ctx.enter_context(tc.tile_pool(name="esp", bufs=4))
    otsp = ctx.enter_context(tc.tile_pool(name="otsp", bufs=2))
    onatp = ctx.enter_context(tc.tile_pool(name="onatp", bufs=2))
    smallp = ctx.enter_context(tc.tile_pool(name="smallp", bufs=4))
    ps_st = ctx.enter_context(tc.tile_pool(name="ps_st", bufs=2, space="PSUM"))
    ps_ot = ctx.enter_context(tc.tile_pool(name="ps_ot", bufs=1, space="PSUM"))
    ps_sm = ctx.enter_context(tc.tile_pool(name="ps_sm", bufs=2, space="PSUM"))

    # constants
    ident = const.tile([128, 128], BF16)
    make_identity(nc, ident)
    ones16 = const.tile([128, 1], BF16)
    nc.vector.memset(ones16, 1.0)
    inv_t = const.tile([128, 1], F32)
    nc.vector.memset(inv_t, 1.0 / T)
    bc2 = const.tile([1, 128], F32)
    nc.vector.memset(bc2, SCALE2)

    std_sb = const.tile([128, T * H // 128], F32)   # [128, 64] contiguous
    nc.sync.dma_start(out=std_sb, in_=std_d.rearrange("(p j) h -> p (j h)", p=128))
    tw_sb = const.tile([1, 1], F32)
    nc.sync.dma_start(out=tw_sb, in_=tw_d)
    tb_sb = const.tile([1, 1], F32)
    nc.sync.dma_start(out=tb_sb, in_=tb_d)
    negw = const.tile([1, 1], F32)
    nc.vector.tensor_scalar_mul(negw, tw_sb, -1.0)
    negb = const.tile([1, 1], F32)
    nc.vector.tensor_scalar_mul(negb, tb_sb, -1.0)

    std3 = std_sb.rearrange("p (j h) -> p j h", h=H)

    # ---- tau prologue (batched over heads: 2 PE matmuls total) ----
    taup = ctx.enter_context(tc.tile_pool(name="taup", bufs=1))
    tau_scs = []

    def emit_taus():
        part_all = smallp.tile([128, H], F32, tag="part")
        for h in range(H):
            nc.vector.reduce_sum(out=part_all[:, h:h + 1], in_=std3[:, :, h], axis=X)
        mean_ps = ps_sm.tile([1, H], F32, tag="ps_sm")
        nc.tensor.matmul(mean_ps, lhsT=inv_t, rhs=part_all, start=True, stop=True)
        ez = smallp.tile([1, H], F32, tag="ez")
        nc.scalar.activation(ez, mean_ps, Exp, bias=negb[:], scale=negw[:])
        den = smallp.tile([1, H], F32, tag="den")
        nc.vector.tensor_scalar_add(den, ez, 1.0)
        sig = smallp.tile([1, H], F32, tag="sig")
        nc.vector.reciprocal(sig, den)
        tau_ps = ps_sm.tile([128, H], F32, tag="ps_sm")
        nc.tensor.matmul(tau_ps, lhsT=bc2, rhs=sig, start=True, stop=True)
        tau_all = taup.tile([128, H], F32)
        nc.vector.tensor_copy(tau_all, tau_ps)
        for h in range(H):
            tau_scs.append(tau_all[:, h:h + 1])

    def prep(h):
        # transposed loads straight from HBM via the DMA crossbar
        qT = qrawp.tile([128, T], BF16, tag="qT")
        nc.sync.dma_start_transpose(qT, q_d[:, h, :])
        kT = big.tile([128, T], BF16, tag="kT")
        nc.sync.dma_start_transpose(kT, k_d[:, h, :])
        v16 = nat.tile([128, TT, 128], BF16, tag="v16")
        nc.sync.dma_start(out=v16, in_=v_d[:, h, :].rearrange("(tt p) d -> p tt d", p=128))

        qsum = smallp.tile([128, 1], F32, tag="qsum")
        nc.vector.reduce_sum(out=qsum, in_=qT, axis=X)
        nmu = smallp.tile([128, 1], F32, tag="nmu")
        nc.vector.tensor_scalar_mul(nmu, qsum, -1.0 / T)
        qcT = big.tile([128, T], BF16, tag="qcT")
        nc.vector.tensor_scalar_add(qcT, qT, nmu)
        return {"qcT": qcT, "kT": kT, "v16": v16}

    def sloop(h, st, lo=0, hi=TT):
        qcT, kT, v16 = st["qcT"], st["kT"], st["v16"]
        tau_sc = tau_scs[h]
        if lo == 0:
            st["ot_ps"] = ps_ot.tile([128, T], F32, tag="ps_ot", name="ot_ps")
            st["esumA"] = esp.tile([128, T], BF16, tag="esumA", name="esumA")
            st["esumB"] = esp.tile([128, T], BF16, tag="esumB", name="esumB")
            st["prev_et"] = None
        ot_ps = st["ot_ps"]
        prev_et = st["prev_et"]
        def emit_av(i, et):
            vlhs = v16[:, i, :]
            nc.tensor.matmul(ot_ps[:, 0:512], lhsT=vlhs, rhs=et[:, 0:512], start=(i == 0), stop=(i == TT - 1))
            nc.tensor.matmul(ot_ps[:, 512:1024], lhsT=vlhs, rhs=et[:, 512:1024], start=(i == 0), stop=(i == TT - 1))

        # in-loop software pipeline: S-matmuls of tile i are emitted before the
        # AV-matmuls of tile i-1, so the PE queue never parks on an AV whose
        # exp hasn't finished while the next S could run.
        pend = st.get("pend_av") or []
        for i in range(lo, hi):
            st_ps = ps_st.tile([128, T], F32, tag="ps_st")
            klhs = kT[:, i * 128:(i + 1) * 128]
            nc.tensor.matmul(st_ps[:, 0:512], lhsT=klhs, rhs=qcT[:, 0:512], start=True, stop=True)
            nc.tensor.matmul(st_ps[:, 512:1024], lhsT=klhs, rhs=qcT[:, 512:1024], start=True, stop=True)
            et = etp.tile([128, T], BF16, tag="et")
            nc.scalar.activation(et, st_ps, Exp, bias=0.0, scale=tau_sc[:])
            pend.append((i, et))
            if len(pend) > 2:
                emit_av(*pend.pop(0))
            # two independent esum chains (halves DVE serial latency and
            # lets the rowsum minis accumulate over both in PSUM)
            esum = st["esumA"] if i < 4 else st["esumB"]
            if i % 4 == 1:
                nc.vector.tensor_add(esum, prev_et, et)
            elif i % 4 > 1:
                nc.vector.tensor_add(esum, esum, et)
            prev_et = et
        if hi == TT:
            while pend:
                emit_av(*pend.pop(0))
        st["pend_av"] = pend
        st["prev_et"] = prev_et

    def finalize(h, st):
        esumA, esumB, ot_ps = st["esumA"], st["esumB"], st["ot_ps"]
        # XBAR output transpose folds t = tt*128 + p (verified), matching the
        # contiguous esum column slices below.
        rs_ps = ps_sm.tile([128, TT], F32, tag="ps_sm")
        for tt in range(TT):
            sl = slice(tt * 128, (tt + 1) * 128)
            nc.tensor.matmul(rs_ps[:, tt:tt + 1], lhsT=esumA[:, sl],
                             rhs=ones16, start=True, stop=False)
            nc.tensor.matmul(rs_ps[:, tt:tt + 1], lhsT=esumB[:, sl],
                             rhs=ones16, start=False, stop=True)
        recipT = smallp.tile([128, TT], F32, tag="recipT")
        nc.vector.reciprocal(recipT, rs_ps)

        ots = otsp.tile([128, T], BF16, tag="ots")
        nc.vector.tensor_copy(ots, ot_ps)
        otp = ps_sm.tile([128, T], BF16, tag="ps_sm")
        for tt in range(TT):
            nc.tensor.transpose(otp[:, tt * 128:(tt + 1) * 128], ots[:, tt * 128:(tt + 1) * 128], ident)
        o_nat = onatp.tile([128, TT, 128], BF16, tag="o_nat")
        for tt in range(TT):
            nc.vector.tensor_scalar_mul(o_nat[:, tt, :], otp[:, tt * 128:(tt + 1) * 128],
                                        recipT[:, tt:tt + 1])
        nc.sync.dma_start(out=o_d[:, h, :].rearrange("(tt p) d -> p tt d", p=128), in_=o_nat)

    # software-pipelined emission, prep depth 2: head h+2's prep (XBAR
    # transposes + centering chain) is issued BEFORE head h's finalize DMAs.
    # The Sync queue is in-order, and finalize's output-transpose blocks on
    # that head's full compute — input transposes must be queued ahead of it
    # or every head transition stalls the Scalar (exp) stream.
    states = [None] * H
    states[0] = prep(0)
    states[1] = prep(1)
    emit_taus()
    sloop(0, states[0])
    for h in range(1, H):
        if h + 1 < H:
            states[h + 1] = prep(h + 1)
        sloop(h, states[h], 0, 1)
        finalize(h - 1, states[h - 1])
        sloop(h, states[h], 1, TT)
    finalize(H - 1, states[H - 1])
    ctx.close()


_BUILT = None


def _build():
    global _BUILT
    if _BUILT is None:
        nc = bacc.Bacc("TRN2", target_bir_lowering=False, debug=False, num_devices=None)
        with tile.TileContext(nc) as tc:
            _emit(tc)
        nc.compile()
        _BUILT = nc
    return _BUILT


def _in_maps(Q, K, V, std, tau_w, tau_b):
    bf = ml_dtypes.bfloat16
    tw = np.asarray(tau_w, np.float32).reshape(1, 1)
    tb = np.asarray(tau_b, np.float32).reshape(1, 1)
    maps = []
    for c in range(NCORES):
        b, n0 = c // 2, (c % 2) * H
        maps.append({
            "Q": np.ascontiguousarray(Q[b, :, n0:n0 + H, :]).astype(bf),
            "K": np.ascontiguousarray(K[b, :, n0:n0 + H, :]).astype(bf),
            "V": np.ascontiguousarray(V[b, :, n0:n0 + H, :]).astype(bf),
            "S": np.ascontiguousarray(std[b, :, n0:n0 + H, 0], np.float32),
            "TW": tw,
            "TB": tb,
        })
    return maps


def _gather(results):
    out = np.empty((B, T, N, D), np.float32)
    for c in range(NCORES):
        b, n0 = c // 2, (c % 2) * H
        out[b, :, n0:n0 + H, :] = np.asarray(results[c]["O"]).astype(np.float32)
    return out


def run(Q, K, V, std, tau_w, tau_b, **spmd_kwargs):
    nc = _build()
    res = run_bass_kernel_spmd(nc, _in_maps(Q, K, V, std, tau_w, tau_b),
                               core_ids=list(range(NCORES)), **spmd_kwargs)
    return _gather(res.results), res


def kernel(Q, K, V, std, tau_w, tau_b):
    out, _ = run(Q, K, V, std, tau_w, tau_b)
    return out
